# revision 61
# baseline (speedup 1.0000x reference)
"""Trainium2 Bass kernel for nn_ClasswiseDBMInnerProduct.

Math (reference):
  nf = feat / ||feat||, nw = weights / ||weights||
  cos = nf @ nw.T                                      [B, C]
  thetas = arccos(cos[i, label_i]) in degrees          [B]
  stats: avg/min/max/stdv of thetas, avg ||w||, avg ||f||
  marginal_logits = 30*(cos(arccos(clip(cos)) + margin_tables) + margin_tables_ext)

Key identity: margin_tables/_ext are zero except at (i, label_i), so
marginal_logits == 30*clip(cos) everywhere except the B label positions,
which get fix_i = 30*(cos(theta_i + add_m_i) + ext_m_i).

Sharding: tensor-parallel over classes. Each of the 8 cores computes a
[B, C/8] slice of cos / logits (columns c in [k*CS, (k+1)*CS)). The label
path (cos[i, label_i], theta stats, margins) is computed redundantly on
every core from an indirect-DMA gather of weights[label] out of the full
weights tensor; each core then scatters fixes for the labels it owns into
its logits slice (out-of-range labels are masked via the indirect-DMA
bounds check). Only avg_w_norm needs cross-core combining: each core
outputs the sum of its shard's weight norms and the host sums 8 scalars.

arccos/cos on device: the ACT Arctan LUT only covers [-pi/2, pi/2], so
  a=|c|, s=sqrt(1-c^2), phi=arctan(min(a,s)/max(a,s)) in [0, pi/4]
  theta+ = phi + (a<=s)*(pi/2-2*phi);  theta = (pi-theta+) + (c>=0)*(2*theta+-pi)
and cos(x) = sin(pi/2 - x) keeps the Sin LUT argument within [-pi, pi].

Performance structure (per core, cost-model driven):
  - output writes dominate (2 x 61 MB); DMAs serialize per issuing engine,
    so the 64 big stage->HBM DMAs are spread over the SP, Pool(SWDGE) and
    ACT queues.
  - matmuls run as float32r (full PE rate for wide moving operands).
  - epilogue: DVE stages cos (PSUM->SBUF copy), ACT stages logits
    (PSUM->SBUF x30); weight/feat prep is emitted first so the main loop
    starts early; the label/theta/margin path is emitted after the main
    loop and fills engine gaps.
"""

import numpy as np

import concourse.bass as bass
import concourse.mybir as mybir
import concourse.tile as tile
import concourse.bass_utils as bass_utils
from concourse import bacc
from concourse.bass import IndirectOffsetOnAxis
from concourse.masks import make_identity

F32 = mybir.dt.float32
F32R = mybir.dt.float32r
I32 = mybir.dt.int32
AX = mybir.AxisListType.X
OP = mybir.AluOpType
ACT = mybir.ActivationFunctionType

P = 128
SCALE = 30.0
RAD2DEG = 180.0 / np.pi
DEG2RAD = np.pi / 180.0
HALF_PI = float(np.pi / 2)

# full-size problem config
B_FULL, D_FULL, C_FULL, NCORES = 4096, 128, 30000, 8

TRACE = False  # set by test.py to profile
N_ACT_DMAS = 0   # big output DMAs routed via the ACT HWDGE ring
# f32r matmul is ~6% faster end-to-end (PE at 1 cycle/row vs 4) but rounds
# inputs to ~1.5e-4 relative; plain fp32 keeps the outputs at ~2e-6 vs the
# reference, which comfortably clears any fp32-envelope absmax gate.
USE_F32R = False
# The fused/sliced DMA variants below are CoreSim-correct but produce wrong
# data on hardware (walrus descriptor-gen divergence, bisected 2026-08-04):
#  - W_SPLIT: sliced 3D-rearranged dma_start halves -> garbage loads
#  - FUSED_GATHER / FUSED_SCATTER: multi-column indirect-DMA offsets
FUSED_GATHER = False
W_SPLIT = False
FUSED_SCATTER = False
STAGE_BUFS = 3


def build_program(B=B_FULL, D=D_FULL, C=C_FULL, ncores=NCORES):
    CS = C // ncores          # classes per core
    RB = B // P               # row blocks
    WT = (CS + P - 1) // P    # weight-shard row tiles
    NBLK = (CS + 511) // 512  # psum col tiles
    BIG = float(2 ** 24 - 1)  # masked-out scatter offset (> any valid index)
    PI = float(np.pi)

    nc = bacc.Bacc("TRN2", target_bir_lowering=False, debug=False)

    feat_d = nc.dram_tensor("feat", [B, D], F32, kind="ExternalInput")
    # wshard is zero-padded on the host to a multiple of P rows
    wsh_d = nc.dram_tensor("wshard", [WT * P, D], F32, kind="ExternalInput")
    wfull_d = nc.dram_tensor("wfull", [C, D], F32, kind="ExternalInput")
    lab_d = nc.dram_tensor("labels", [B, 1], I32, kind="ExternalInput")
    c0_d = nc.dram_tensor("c0", [1, 1], F32, kind="ExternalInput")
    cos_d = nc.dram_tensor("cos_out", [B, CS], F32, kind="ExternalOutput")
    # logits split into row chunks (host concatenates): each chunk's label
    # scatters then only wait on that chunk's bulk DMAs instead of the whole
    # tensor's last write, hiding most of the scatter latency mid-kernel.
    NCH = 4 if RB % 4 == 0 else (2 if RB % 2 == 0 else 1)
    CHB = RB // NCH                 # blocks per chunk
    CHR = CHB * P                   # rows per chunk
    log_ds = [
        nc.dram_tensor(f"logits_out{q}", [CHR, CS], F32, kind="ExternalOutput")
        for q in range(NCH)
    ]
    stats_d = nc.dram_tensor("stats_out", [1, 16], F32, kind="ExternalOutput")

    feat_ap = feat_d.ap()
    wsh_ap = wsh_d.ap()
    wfull_ap = wfull_d.ap()
    lab_ap = lab_d.ap()
    cos_ap = cos_d.ap()
    log_aps = [d.ap() for d in log_ds]
    log_flats = [ap.rearrange("a b -> (a b)")[:, None] for ap in log_aps]

    with tile.TileContext(nc) as tc:
        with (
            tc.tile_pool(name="persist", bufs=1) as persist,
            tc.tile_pool(name="io", bufs=3) as io,
            tc.tile_pool(name="scratch", bufs=2) as scratch,
            tc.tile_pool(name="stage", bufs=STAGE_BUFS) as stage,
            tc.tile_pool(name="tpsum", bufs=1, space="PSUM") as tpsum,
            tc.tile_pool(name="spsum", bufs=1, space="PSUM") as spsum,
            tc.tile_pool(name="mpsum", bufs=6, space="PSUM") as mpsum,
        ):
            ident = persist.tile([P, P], F32, tag="ident")
            make_identity(nc, ident[:])
            ones_row = persist.tile([1, P], F32, tag="ones_row")
            nc.vector.memset(ones_row[:], 1.0)
            ones_col = persist.tile([P, 1], F32, tag="ones_col")
            nc.vector.memset(ones_col[:], 1.0)
            halfpi_col = persist.tile([P, 1], F32, tag="halfpi_col")
            nc.vector.memset(halfpi_col[:], HALF_PI)

            # labels as [P, RB]: element (r, b) = label[b*P + r]
            lab_i = persist.tile([P, RB], I32, tag="lab_i")
            nc.sync.dma_start(
                out=lab_i[:], in_=lab_ap.rearrange("(b r) one -> r (b one)", r=P)
            )
            c0_s = persist.tile([1, 1], F32, tag="c0_s")
            nc.sync.dma_start(out=c0_s[:], in_=c0_d.ap())

            # persistent matmul operands; float32r so the PE runs at full
            # rate (the staging copies round f32 -> f32r, as the BIR
            # verifier requires for f32r matmul inputs)
            mm_dt = F32R if USE_F32R else F32
            nfT = persist.tile([P, B], mm_dt, tag="nfT")    # raw feat^T [D, B]
            nwT = persist.tile([P, CS], mm_dt, tag="nwT")   # normalized wshard^T [D, CS]

            # per-row stats tiles [P, RB]
            fnorm = persist.tile([P, RB], F32, tag="fnorm")
            frinv = persist.tile([P, RB], F32, tag="frinv")
            fss = persist.tile([P, RB], F32, tag="fss")
            wlss = persist.tile([P, RB], F32, tag="wlss")
            dotf = persist.tile([P, RB], F32, tag="dotf")
            wnorms = persist.tile([P, WT], F32, tag="wnorms")
            nc.vector.memset(wnorms[:], 0.0)

            # Bulk input loads: one DMA each for feat / wshard(padded) and one
            # fused indirect gather for all weights[label] rows. Layout
            # [P, nblocks, D]: partition = row-within-block.
            # w_all first on SP (phase B gates the whole main loop); feat on
            # the otherwise-idle ACT ring; the gather on Pool.
            w_all = persist.tile([P, WT, D], F32, tag="w_all")
            wsh_r = wsh_ap.rearrange("(t r) d -> r t d", r=P)
            if W_SPLIT:
                wh = WT // 2
                nc.sync.dma_start(out=w_all[:, :wh, :], in_=wsh_r[:, :wh, :])
                nc.sync.dma_start(out=w_all[:, wh:, :], in_=wsh_r[:, wh:, :])
            else:
                nc.sync.dma_start(out=w_all[:], in_=wsh_r)
            # f_all on the Pool queue: ACT must start its w squares at once
            # (they gate nwT and so the whole main loop), and the Pool
            # gathers are deferrable gap-fill work
            f_all = persist.tile([P, RB, D], F32, tag="f_all")
            nc.gpsimd.dma_start(
                out=f_all[:], in_=feat_ap.rearrange("(b r) d -> r b d", r=P)
            )
            wl_all = persist.tile([P, RB, D], F32, tag="wl_all")
            if FUSED_GATHER:
                nc.gpsimd.indirect_dma_start(
                    out=wl_all[:],
                    out_offset=None,
                    in_=wfull_ap,
                    in_offset=IndirectOffsetOnAxis(ap=lab_i[:, :], axis=0),
                )
            else:
                for b in range(RB):
                    nc.gpsimd.indirect_dma_start(
                        out=wl_all[:, b, :],
                        out_offset=None,
                        in_=wfull_ap,
                        in_offset=IndirectOffsetOnAxis(ap=lab_i[:, b : b + 1], axis=0),
                    )

            # ---------------- phase B: weight shard prep (first: the main
            # loop's first stage needs ALL of nwT) ----------------
            # ---------------- phase A1: transpose RAW feat blocks (PE/DVE only;
            # normalization is folded into the epilogue row scales, so the
            # main loop isn't gated on any norm computation for feat) --------
            for b in range(RB):
                rows = slice(b * P, (b + 1) * P)
                tp = tpsum.tile([P, P], F32, tag="tp", space="PSUM")
                nc.tensor.transpose(tp[:], f_all[:, b, :], ident[:])
                nc.vector.tensor_copy(nfT[:, rows], tp[:])

            wss_all = persist.tile([P, WT], F32, tag="wss_all")
            for t in range(WT):
                sqw_s = scratch.tile([P, D], F32, tag="sqw_s")
                nc.scalar.activation(
                    sqw_s[:], w_all[:, t, :], ACT.Square, accum_out=wss_all[:, t : t + 1]
                )
            # one batched sqrt/recip for all tiles; pad rows (beyond CS) have
            # norm 0 -> clamp before reciprocal so nw stays finite (0*1e20=0)
            # and wnorms stays 0 there, keeping the wsum stat exact (the host
            # zero-pads wshard to WT*P rows).
            nc.scalar.sqrt(wnorms[:], wss_all[:])
            wri_all = persist.tile([P, WT], F32, tag="wri_all")
            nc.vector.tensor_scalar(wri_all[:], wnorms[:], 1e-20, None, OP.max)
            nc.vector.reciprocal(wri_all[:], wri_all[:])
            for t in range(WT):
                rows = min(P, CS - t * P)
                rsl = slice(t * P, t * P + rows)
                nw_t = io.tile([P, D], F32, tag="nw_t")
                nc.vector.tensor_scalar_mul(nw_t[:], w_all[:, t, :], wri_all[:, t : t + 1])
                tp = tpsum.tile([P, P], F32, tag="tp", space="PSUM")
                nc.tensor.transpose(tp[:, :rows], nw_t[:rows], ident[:rows, :rows])
                nc.vector.tensor_copy(nwT[:, rsl], tp[:, :rows])



            # f norms after the B-phase in priority order (they only gate the
            # G epilogue scales); squares on the idle Pool engine, reduce DVE
            for b in range(RB):
                sq_s = scratch.tile([P, D], F32, tag="sq_s")
                nc.gpsimd.tensor_tensor(
                    sq_s[:], f_all[:, b, :], f_all[:, b, :], OP.mult
                )
                nc.vector.reduce_sum(fss[:, b : b + 1], sq_s[:], axis=AX)
            nc.scalar.sqrt(fnorm[:], fss[:])
            nc.vector.reciprocal(frinv[:], fnorm[:])
            frinv30 = persist.tile([P, RB], F32, tag="frinv30")
            nc.vector.tensor_scalar_mul(frinv30[:], frinv[:], SCALE)

            # ---------------- phase G: main matmul + bulk outputs ----------------
            # f32r runs the PE at full rate for wide moving operands (plain
            # f32 is 4 cycles/row).
            nfT_r = nfT[:]
            nwT_r = nwT[:]
            # DMAs serialize on the issuing engine's queue; SP / Pool take the
            # bulk, ACT (busy with the logits-scale pass) only N_ACT_DMAS.
            # weighted split: Pool also runs the per-block gathers+scatters,
            # ACT the logits-scale pass, so SP takes the largest share
            shares = [(nc.sync, 17), (nc.gpsimd, 13), (nc.scalar, 2)]
            total_w = sum(w for _, w in shares)
            dma_engines = []
            acc = [0.0] * len(shares)
            for _ in range(2 * RB):
                for j, (_, w) in enumerate(shares):
                    acc[j] += w
                k = max(range(len(shares)), key=lambda j: acc[j])
                acc[k] -= total_w
                dma_engines.append(shares[k][0])
            for b in range(RB):
                rows = slice(b * P, (b + 1) * P)
                q = b // CHB
                qrows = slice((b - q * CHB) * P, (b - q * CHB + 1) * P)
                cs_t = stage.tile([P, CS], F32, tag="cs_t")
                lg_t = stage.tile([P, CS], F32, tag="lg_t")
                for j in range(NBLK):
                    w = min(512, CS - j * 512)
                    cols = slice(j * 512, j * 512 + w)
                    mp = mpsum.tile([P, 512], F32, tag="mp", space="PSUM")
                    nc.tensor.matmul(
                        out=mp[:, :w],
                        lhsT=nfT_r[:, rows],
                        rhs=nwT_r[:, cols],
                        start=True,
                        stop=True,
                    )
                    # row scale by 1/|f| (and x30 for logits) finishes the
                    # feat normalization in the same pass that stages PSUM
                    nc.vector.tensor_scalar_mul(
                        cs_t[:, cols], mp[:, :w], frinv[:, b : b + 1]
                    )
                    nc.scalar.mul(lg_t[:, cols], mp[:, :w], frinv30[:, b : b + 1])
                if b == 0 and NBLK >= 2:
                    # split the first block's writes so the output stream
                    # starts as soon as the first half of the stage is ready
                    h = (NBLK // 2) * 512
                    dma_engines[0].dma_start(out=cos_ap[rows, :h], in_=cs_t[:, :h])
                    dma_engines[0].dma_start(out=cos_ap[rows, h:], in_=cs_t[:, h:])
                    dma_engines[1].dma_start(out=log_aps[q][qrows, :h], in_=lg_t[:, :h])
                    dma_engines[1].dma_start(out=log_aps[q][qrows, h:], in_=lg_t[:, h:])
                else:
                    dma_engines[2 * b].dma_start(out=cos_ap[rows, :], in_=cs_t[:])
                    dma_engines[2 * b + 1].dma_start(out=log_aps[q][qrows, :], in_=lg_t[:])

            # ---------------- phase A2: label-gather dot products (gap-fill) ----
            for b in range(RB):
                wl_t = wl_all[:, b, :]
                sq2_s = scratch.tile([P, D], F32, tag="sq2_s")
                nc.scalar.activation(
                    sq2_s[:], wl_t, ACT.Square, accum_out=wlss[:, b : b + 1]
                )
                pr_s = scratch.tile([P, D], F32, tag="pr_s")
                nc.vector.tensor_tensor(pr_s[:], f_all[:, b, :], wl_t, OP.mult)
                nc.vector.reduce_sum(dotf[:, b : b + 1], pr_s[:], axis=AX)

            # ---------------- phase D: cos_l and thetas (fills engine gaps;
            # ACT ops ordered to minimize activation-table swaps) ----------------
            wlno = persist.tile([P, RB], F32, tag="wlno")
            nc.scalar.sqrt(wlno[:], wlss[:])               # sqrt table
            wlri = persist.tile([P, RB], F32, tag="wlri")
            nc.vector.reciprocal(wlri[:], wlno[:])
            cosl = persist.tile([P, RB], F32, tag="cosl")
            nc.vector.tensor_tensor(cosl[:], dotf[:], frinv[:], OP.mult)
            nc.vector.tensor_tensor(cosl[:], cosl[:], wlri[:], OP.mult)
            nc.vector.tensor_scalar(cosl[:], cosl[:], -1.0, 1.0, OP.max, OP.min)
            csq = scratch.tile([P, RB], F32, tag="csq")
            nc.scalar.square(csq[:], cosl[:])              # sqrt table has Square
            a_t = scratch.tile([P, RB], F32, tag="a_t")    # |c| = sqrt(c^2)
            nc.scalar.sqrt(a_t[:], csq[:])
            nc.vector.tensor_scalar(csq[:], csq[:], -1.0, 1.0, OP.mult, OP.add)
            nc.vector.tensor_scalar(csq[:], csq[:], 0.0, None, OP.max)
            s_t = scratch.tile([P, RB], F32, tag="s_t")
            nc.scalar.sqrt(s_t[:], csq[:])                 # sqrt table
            mn_t = scratch.tile([P, RB], F32, tag="mn_t")
            nc.vector.tensor_tensor(mn_t[:], a_t[:], s_t[:], OP.min)
            mx_t = scratch.tile([P, RB], F32, tag="mx_t")
            nc.vector.tensor_tensor(mx_t[:], a_t[:], s_t[:], OP.max)
            nc.vector.reciprocal(mx_t[:], mx_t[:])         # max(a,s) >= 1/sqrt(2)
            q_t = scratch.tile([P, RB], F32, tag="q_t")
            nc.vector.tensor_tensor(q_t[:], mn_t[:], mx_t[:], OP.mult)
            phi = scratch.tile([P, RB], F32, tag="phi")
            nc.scalar.activation(phi[:], q_t[:], ACT.Arctan)  # -> trig table
            c1_t = scratch.tile([P, RB], F32, tag="c1_t")
            nc.vector.tensor_tensor(c1_t[:], a_t[:], s_t[:], OP.is_le)
            t1_t = scratch.tile([P, RB], F32, tag="t1_t")
            nc.vector.tensor_scalar(t1_t[:], phi[:], -2.0, HALF_PI, OP.mult, OP.add)
            nc.vector.tensor_tensor(t1_t[:], c1_t[:], t1_t[:], OP.mult)
            thp = scratch.tile([P, RB], F32, tag="thp")
            nc.vector.tensor_tensor(thp[:], phi[:], t1_t[:], OP.add)
            c2_t = scratch.tile([P, RB], F32, tag="c2_t")
            nc.vector.tensor_scalar(c2_t[:], cosl[:], 0.0, None, OP.is_ge)
            u1_t = scratch.tile([P, RB], F32, tag="u1_t")
            nc.vector.tensor_scalar(u1_t[:], thp[:], 2.0, -PI, OP.mult, OP.add)
            nc.vector.tensor_tensor(u1_t[:], c2_t[:], u1_t[:], OP.mult)
            th_rad = persist.tile([P, RB], F32, tag="th_rad")
            nc.vector.tensor_scalar(th_rad[:], thp[:], -1.0, PI, OP.mult, OP.add)
            nc.vector.tensor_tensor(th_rad[:], th_rad[:], u1_t[:], OP.add)
            th_deg = persist.tile([P, RB], F32, tag="th_deg")
            nc.vector.tensor_scalar_mul(th_deg[:], th_rad[:], float(RAD2DEG))

            # ---------------- phase S1: partition-reduced scalars ----------------
            red = persist.tile([P, 8], F32, tag="red")
            nc.vector.memset(red[:], 0.0)
            nc.vector.reduce_sum(red[:, 0:1], th_deg[:], axis=AX)
            nc.vector.reduce_max(red[:, 1:2], th_deg[:], axis=AX)
            nc.vector.tensor_reduce(red[:, 2:3], th_deg[:], axis=AX, op=OP.min)
            nc.vector.reduce_sum(red[:, 3:4], fnorm[:], axis=AX)
            nc.vector.reduce_sum(red[:, 4:5], wnorms[:], axis=AX)

            def part_reduce(src_col, op, tag):
                """[P,1] -> [1,1] scalar via PE transpose + free-axis reduce."""
                tp_r = spsum.tile([1, P], F32, tag="sp", space="PSUM", name="sp_tr")[:, :]
                nc.tensor.transpose(tp_r, src_col, ident[:])
                out = persist.tile([1, 1], F32, tag=tag)
                nc.vector.tensor_reduce(out[:], tp_r[:], axis=AX, op=op)
                return out

            tsum = part_reduce(red[:, 0:1], OP.add, "tsum")
            tmax = part_reduce(red[:, 1:2], OP.max, "tmax")
            tmin = part_reduce(red[:, 2:3], OP.min, "tmin")
            xsum = part_reduce(red[:, 3:4], OP.add, "xsum")
            wsum = part_reduce(red[:, 4:5], OP.add, "wsum")
            tavg = persist.tile([1, 1], F32, tag="tavg")
            nc.vector.tensor_scalar_mul(tavg[:], tsum[:], 1.0 / B)

            # margin_above = (max<90) * (90-avg)*DEG2RAD  (scalar)
            ma1 = persist.tile([1, 1], F32, tag="ma1")
            nc.vector.tensor_scalar(ma1[:], tmax[:], 90.0, None, OP.is_lt)
            ma2 = persist.tile([1, 1], F32, tag="ma2")
            nc.vector.tensor_scalar(
                ma2[:], tavg[:], -float(DEG2RAD), 90.0 * float(DEG2RAD), OP.mult, OP.add
            )
            ma = persist.tile([1, 1], F32, tag="ma")
            nc.vector.tensor_tensor(ma[:], ma1[:], ma2[:], OP.mult)

            # broadcast [avg, ma, c0] across partitions via K=1 matmul
            bsrc = persist.tile([1, 4], F32, tag="bsrc")
            nc.vector.memset(bsrc[:], 0.0)
            nc.vector.tensor_copy(bsrc[:, 0:1], tavg[:])
            nc.vector.tensor_copy(bsrc[:, 1:2], ma[:])
            nc.vector.tensor_copy(bsrc[:, 2:3], c0_s[:])
            bps = spsum.tile([P, 4], F32, tag="sp", space="PSUM", name="sp_bc")[:, :]
            nc.tensor.matmul(
                out=bps, lhsT=ones_row[:], rhs=bsrc[:], start=True, stop=True
            )
            bcast = persist.tile([P, 4], F32, tag="bcast")
            nc.vector.tensor_copy(bcast[:], bps)
            avg_b = bcast[:, 0:1]
            ma_b = bcast[:, 1:2]
            c0_b = bcast[:, 2:3]

            # ---------------- phase E: margins + fix values [P, RB] ----------------
            tm = persist.tile([P, RB], F32, tag="tm")
            nc.vector.tensor_scalar(tm[:], th_deg[:], avg_b, None, OP.is_gt)
            nc.vector.tensor_scalar_mul(tm[:], tm[:], ma_b)
            cosm = scratch.tile([P, RB], F32, tag="cosm")
            # cos(tm) = sin(pi/2 - tm)
            nc.scalar.activation(cosm[:], tm[:], ACT.Sin, bias=halfpi_col[:], scale=-1.0)
            nc.vector.tensor_scalar_mul(cosm[:], cosm[:], -1.0)
            cond = scratch.tile([P, RB], F32, tag="cond")
            nc.vector.tensor_tensor(cond[:], cosl[:], cosm[:], OP.is_gt)
            addm = scratch.tile([P, RB], F32, tag="addm")
            nc.vector.tensor_tensor(addm[:], cond[:], tm[:], OP.mult)
            sintm = scratch.tile([P, RB], F32, tag="sintm")
            nc.scalar.activation(sintm[:], tm[:], ACT.Sin)
            nc.vector.tensor_tensor(sintm[:], tm[:], sintm[:], OP.mult)
            nc.vector.tensor_scalar_mul(sintm[:], sintm[:], -1.0)  # -tm*sin(tm)
            notc = scratch.tile([P, RB], F32, tag="notc")
            nc.vector.tensor_scalar(notc[:], cond[:], -1.0, 1.0, OP.mult, OP.add)
            extm = scratch.tile([P, RB], F32, tag="extm")
            nc.vector.tensor_tensor(extm[:], notc[:], sintm[:], OP.mult)
            arg = scratch.tile([P, RB], F32, tag="arg")
            nc.vector.tensor_tensor(arg[:], th_rad[:], addm[:], OP.add)
            fix = persist.tile([P, RB], F32, tag="fix")
            # cos(arg) = sin(pi/2 - arg); arg in [0, 3pi/2] -> sin arg in [-pi, pi/2]
            nc.scalar.activation(fix[:], arg[:], ACT.Sin, bias=halfpi_col[:], scale=-1.0)
            nc.vector.tensor_tensor(fix[:], fix[:], extm[:], OP.add)
            nc.vector.tensor_scalar_mul(fix[:], fix[:], SCALE)

            # stdv: sum((theta-avg)^2) / (B-1), partition-sum via ones matmul
            dif = scratch.tile([P, RB], F32, tag="dif")
            nc.vector.tensor_scalar(dif[:], th_deg[:], avg_b, None, OP.subtract)
            dsq = scratch.tile([P, RB], F32, tag="dsq")
            dcol = persist.tile([P, 1], F32, tag="dcol")
            nc.scalar.activation(dsq[:], dif[:], ACT.Square, accum_out=dcol[:])
            vps = spsum.tile([1, 1], F32, tag="sp", space="PSUM", name="sp_var")[:, :]
            nc.tensor.matmul(
                out=vps, lhsT=ones_col[:], rhs=dcol[:], start=True, stop=True
            )
            stv = persist.tile([1, 1], F32, tag="stv")
            nc.vector.tensor_scalar_mul(stv[:], vps, 1.0 / (B - 1))
            nc.scalar.sqrt(stv[:], stv[:])

            # stats_out: [avg, min, max, stdv, wsum, xsum, 0...]
            st_t = persist.tile([1, 16], F32, tag="st_t")
            nc.vector.memset(st_t[:], 0.0)
            nc.vector.tensor_copy(st_t[:, 0:1], tavg[:])
            nc.vector.tensor_copy(st_t[:, 1:2], tmin[:])
            nc.vector.tensor_copy(st_t[:, 2:3], tmax[:])
            nc.vector.tensor_copy(st_t[:, 3:4], stv[:])
            nc.vector.tensor_copy(st_t[:, 4:5], wsum[:])
            nc.vector.tensor_copy(st_t[:, 5:6], xsum[:])
            nc.sync.dma_start(out=stats_d.ap(), in_=st_t[:])

            # ---------------- phase F: scatter offsets ----------------
            g_i = persist.tile([P, RB], I32, tag="g_i")
            nc.gpsimd.iota(g_i[:], pattern=[[P, RB]], base=0, channel_multiplier=1)
            g_f = persist.tile([P, RB], F32, tag="g_f")
            nc.vector.tensor_copy(g_f[:], g_i[:])
            lab_f = persist.tile([P, RB], F32, tag="lab_f")
            nc.vector.tensor_copy(lab_f[:], lab_i[:])
            d_f = persist.tile([P, RB], F32, tag="d_f")
            nc.vector.tensor_scalar(d_f[:], lab_f[:], c0_b, None, OP.subtract)
            v1 = scratch.tile([P, RB], F32, tag="v1")
            nc.vector.tensor_scalar(v1[:], d_f[:], 0.0, None, OP.is_ge)
            v2 = scratch.tile([P, RB], F32, tag="v2")
            nc.vector.tensor_scalar(v2[:], d_f[:], float(CS), None, OP.is_lt)
            nc.vector.tensor_tensor(v1[:], v1[:], v2[:], OP.mult)
            nc.vector.tensor_scalar(d_f[:], d_f[:], 0.0, float(CS - 1), OP.max, OP.min)
            off_f = persist.tile([P, RB], F32, tag="off_f")
            nc.vector.tensor_scalar_mul(off_f[:], g_f[:], float(CS))
            nc.vector.tensor_tensor(off_f[:], off_f[:], d_f[:], OP.add)
            nc.vector.tensor_tensor(off_f[:], off_f[:], v1[:], OP.mult)
            # + (1-v)*BIG
            nc.vector.tensor_scalar(v1[:], v1[:], -BIG, BIG, OP.mult, OP.add)
            nc.vector.tensor_tensor(off_f[:], off_f[:], v1[:], OP.add)
            # per-chunk offsets: subtract the chunk base so indices address the
            # chunk tensor; masked entries stay above the chunk bounds check
            # (BIG - (NCH-1)*CHR*CS > CHR*CS - 1 for these sizes).
            off_i = persist.tile([P, RB], I32, tag="off_i")
            for q in range(NCH):
                bsl = slice(q * CHB, (q + 1) * CHB)
                nc.vector.tensor_scalar(
                    off_f[:, bsl], off_f[:, bsl], -float(q * CHB * P * CS), None, OP.add
                )
            nc.vector.tensor_copy(off_i[:], off_f[:])

            # ---------------- phase H: label fix scatter (per chunk, after that
            # chunk's bulk writes only) ----------------
            for b in range(RB):
                q = b // CHB
                nc.gpsimd.indirect_dma_start(
                    out=log_flats[q],
                    out_offset=IndirectOffsetOnAxis(ap=off_i[:, b : b + 1], axis=0),
                    in_=fix[:, b : b + 1],
                    in_offset=None,
                    bounds_check=CHB * P * CS - 1,
                    oob_is_err=False,
                )

    nc.compile()
    return nc


_PROGRAM_CACHE = {}


def _get_program(key):
    if key not in _PROGRAM_CACHE:
        _PROGRAM_CACHE[key] = build_program(*key)
    return _PROGRAM_CACHE[key]


def make_in_maps(feat, label, weights, B, D, C, ncores):
    CS = C // ncores
    WT = (CS + P - 1) // P
    feat = np.ascontiguousarray(np.asarray(feat, dtype=np.float32))
    weights = np.ascontiguousarray(np.asarray(weights, dtype=np.float32))
    lab32 = np.ascontiguousarray(np.asarray(label).astype(np.int32).reshape(B, 1))
    in_maps = []
    for k in range(ncores):
        wsh = np.zeros((WT * P, D), np.float32)
        wsh[:CS] = weights[k * CS : (k + 1) * CS]
        in_maps.append(
            {
                "feat": feat,
                "wshard": wsh,
                "wfull": weights,
                "labels": lab32,
                "c0": np.array([[k * CS]], dtype=np.float32),
            }
        )
    return in_maps


def kernel(feat, label, weights):
    B, D = feat.shape
    C = weights.shape[0]
    ncores = NCORES
    CS = C // ncores
    nc = _get_program((B, D, C, ncores))
    in_maps = make_in_maps(feat, label, weights, B, D, C, ncores)
    res = bass_utils.run_bass_kernel_spmd(
        nc, in_maps, core_ids=list(range(ncores)), trace=TRACE
    )
    outs = res.results
    RB = B // P
    NCH = 4 if RB % 4 == 0 else (2 if RB % 2 == 0 else 1)
    cos = np.concatenate([outs[k]["cos_out"] for k in range(ncores)], axis=1)
    logits = np.concatenate(
        [
            np.concatenate([outs[k][f"logits_out{q}"] for q in range(NCH)], axis=0)
            for k in range(ncores)
        ],
        axis=1,
    )
    st0 = outs[0]["stats_out"][0]
    avg_theta = np.float32(st0[0])
    min_theta = np.float32(st0[1])
    max_theta = np.float32(st0[2])
    stdv_theta = np.float32(st0[3])
    avg_w_norm = np.float32(
        sum(outs[k]["stats_out"][0, 4] for k in range(ncores)) / C
    )
    avg_x_norm = np.float32(st0[5] / B)
    if TRACE:
        kernel.last_results = res
    return (cos, logits, avg_theta, min_theta, max_theta, stdv_theta,
            avg_w_norm, avg_x_norm)
